# revision 1
# baseline (speedup 1.0000x reference)
"""Trainium2 Bass kernel for nn_PopulationSNN: 3-layer LIF SNN, T=100 timesteps.

Sharding: data-parallel over batch — 2048 rows split across 8 NeuronCores
(256 rows each); weights replicated; the sequential timestep scan runs
independently per shard.

Per-core kernel structure:
  - "phase A": per step t, g1 = x_t @ (0.5*W1).T + 0.5*b1 on TensorE (bf16
    operands, fp32 PSUM accumulate), double-buffered one step ahead of the scan.
  - LIF scan (per layer): u = 0.5*y + g ; y' = (u<1)*u ; spike mask s=(u>=1)
    emitted as bf16 and fed directly as the next layer's matmul rhs.
  - Output: spike counts accumulated on-chip; divided by T on host.
"""

from contextlib import ExitStack

import numpy as np
import ml_dtypes

import concourse.bacc as bacc
import concourse.mybir as mybir
import concourse.tile as tile
from concourse import bass_utils
from concourse._compat import with_exitstack

FP32 = mybir.dt.float32
BF16 = mybir.dt.bfloat16
ALU = mybir.AluOpType

B_FULL, NIN, T_FULL = 2048, 512, 100
H1, H2, O = 512, 256, 5
OP = 8          # O padded for PSUM/DMA friendliness
NCORES = 8
BS = B_FULL // NCORES  # 256 per-core batch


def _build_snn(T: int = T_FULL, B: int = BS):
    nc = bacc.Bacc(trn_type="TRN2")
    x_d = nc.dram_tensor("x", [NIN, T, B], BF16, kind="ExternalInput")
    w1t_d = nc.dram_tensor("w1t", [NIN, H1], BF16, kind="ExternalInput")
    b1_d = nc.dram_tensor("b1", [1, H1], BF16, kind="ExternalInput")
    w2t_d = nc.dram_tensor("w2t", [H1, H2], BF16, kind="ExternalInput")
    b2_d = nc.dram_tensor("b2", [1, H2], BF16, kind="ExternalInput")
    wot_d = nc.dram_tensor("wot", [H2, OP], BF16, kind="ExternalInput")
    bo_d = nc.dram_tensor("bo", [1, OP], BF16, kind="ExternalInput")
    acc_d = nc.dram_tensor("acc", [OP, B], FP32, kind="ExternalOutput")

    with tile.TileContext(nc) as tc:
        _snn_body(tc, x_d, w1t_d, b1_d, w2t_d, b2_d, wot_d, bo_d, acc_d, T, B)
    nc.compile()
    return nc


@with_exitstack
def _snn_body(ctx: ExitStack, tc, x_d, w1t_d, b1_d, w2t_d, b2_d, wot_d, bo_d,
              acc_d, T, B):
    nc = tc.nc
    K1, M1 = 4, 4   # layer-1 K tiles / M tiles
    K2, M2 = 4, 2
    K3 = 2

    consts = ctx.enter_context(tc.tile_pool(name="consts", bufs=1))
    xin = ctx.enter_context(tc.tile_pool(name="xin", bufs=4))
    state = ctx.enter_context(tc.tile_pool(name="state", bufs=1))
    work = ctx.enter_context(tc.tile_pool(name="work", bufs=2))
    masks = ctx.enter_context(tc.tile_pool(name="masks", bufs=2))
    psum_g1 = ctx.enter_context(tc.tile_pool(name="psum_g1", bufs=2, space="PSUM"))
    psum_h2 = ctx.enter_context(tc.tile_pool(name="psum_h2", bufs=2, space="PSUM"))
    psum_h3 = ctx.enter_context(tc.tile_pool(name="psum_h3", bufs=2, space="PSUM"))

    w1_sb = []
    for k in range(K1):
        t_ = consts.tile([128, H1], BF16, tag=f"w1_{k}")
        nc.sync.dma_start(t_[:], w1t_d[k * 128:(k + 1) * 128, :])
        w1_sb.append(t_)
    w2_sb = []
    for k in range(K2):
        t_ = consts.tile([128, H2], BF16, tag=f"w2_{k}")
        nc.sync.dma_start(t_[:], w2t_d[k * 128:(k + 1) * 128, :])
        w2_sb.append(t_)
    wo_sb = []
    for k in range(K3):
        t_ = consts.tile([128, OP], BF16, tag=f"wo_{k}")
        nc.sync.dma_start(t_[:], wot_d[k * 128:(k + 1) * 128, :])
        wo_sb.append(t_)
    b1_sb = consts.tile([1, H1], BF16, tag="b1")
    nc.sync.dma_start(b1_sb[:], b1_d[:, :])
    b2_sb = consts.tile([1, H2], BF16, tag="b2")
    nc.sync.dma_start(b2_sb[:], b2_d[:, :])
    bo_sb = consts.tile([1, OP], BF16, tag="bo")
    nc.sync.dma_start(bo_sb[:], bo_d[:, :])
    ones_sb = consts.tile([1, B], BF16, tag="ones")
    nc.vector.memset(ones_sb[:], 1.0)

    y1 = state.tile([128, M1 * B], FP32, tag="y1")
    y2 = state.tile([128, M2 * B], FP32, tag="y2")
    y3 = state.tile([OP, B], FP32, tag="y3")
    acc = state.tile([OP, B], FP32, tag="acc")
    nc.vector.memset(y1[:], 0.0)
    nc.vector.memset(y2[:], 0.0)
    nc.vector.memset(y3[:], 0.0)
    nc.vector.memset(acc[:], 0.0)

    def load_x(t):
        tiles = []
        for k in range(K1):
            xt = xin.tile([128, B], BF16, tag=f"x_{k}")
            nc.sync.dma_start(xt[:], x_d[k * 128:(k + 1) * 128, t, :])
            tiles.append(xt)
        return tiles

    def phase_a(t, x_tiles):
        g1 = psum_g1.tile([128, M1 * B], FP32, tag="g1")
        for m in range(M1):
            out = g1[:, m * B:(m + 1) * B]
            nc.tensor.matmul(out, b1_sb[0:1, m * 128:(m + 1) * 128],
                             ones_sb[0:1, :], start=True, stop=False)
            for k in range(K1):
                nc.tensor.matmul(out, w1_sb[k][:, m * 128:(m + 1) * 128],
                                 x_tiles[k][:], start=False, stop=(k == K1 - 1))
        return g1

    def lif(y, g, width):
        u = work.tile([y.shape[0], width], FP32, tag=f"u_{width}")
        nc.vector.scalar_tensor_tensor(u[:, :], y[:, :width], 0.5, g[:, :width],
                                       op0=ALU.mult, op1=ALU.add)
        nc.vector.scalar_tensor_tensor(y[:, :width], u[:, :], 1.0, u[:, :],
                                       op0=ALU.is_lt, op1=ALU.mult)
        return u

    prev_g1 = phase_a(0, load_x(0))

    for t in range(T):
        g1 = prev_g1
        if t + 1 < T:
            prev_g1 = phase_a(t + 1, load_x(t + 1))

        u1 = lif(y1, g1, M1 * B)
        m1 = masks.tile([128, M1 * B], BF16, tag="m1")
        nc.vector.tensor_scalar(m1[:], u1[:], 1.0, None, op0=ALU.is_ge)

        h2 = psum_h2.tile([128, M2 * B], FP32, tag="h2")
        for m in range(M2):
            out = h2[:, m * B:(m + 1) * B]
            nc.tensor.matmul(out, b2_sb[0:1, m * 128:(m + 1) * 128],
                             ones_sb[0:1, :], start=True, stop=False)
            for k in range(K2):
                nc.tensor.matmul(out, w2_sb[k][:, m * 128:(m + 1) * 128],
                                 m1[:, k * B:(k + 1) * B], start=False,
                                 stop=(k == K2 - 1))

        u2 = lif(y2, h2, M2 * B)
        m2 = masks.tile([128, M2 * B], BF16, tag="m2")
        nc.vector.tensor_scalar(m2[:], u2[:], 1.0, None, op0=ALU.is_ge)

        h3 = psum_h3.tile([OP, B], FP32, tag="h3")
        nc.tensor.matmul(h3[:, :], bo_sb[0:1, :], ones_sb[0:1, :],
                         start=True, stop=False)
        for k in range(K3):
            nc.tensor.matmul(h3[:, :], wo_sb[k][:, :], m2[:, k * B:(k + 1) * B],
                             start=False, stop=(k == K3 - 1))

        u3 = lif(y3, h3, B)
        nc.vector.scalar_tensor_tensor(acc[:, :], u3[:, :], 1.0, acc[:, :],
                                       op0=ALU.is_ge, op1=ALU.add)

    nc.sync.dma_start(acc_d[:, :], acc[:, :])


_NC_CACHE = {}


def _get_nc(T, B):
    key = (T, B)
    if key not in _NC_CACHE:
        _NC_CACHE[key] = _build_snn(T, B)
    return _NC_CACHE[key]


def _prep_in_maps(x, W1, b1, W2, b2, Wo, bo):
    bf = ml_dtypes.bfloat16
    base = {
        "w1t": np.ascontiguousarray((0.5 * W1.astype(np.float32)).T).astype(bf),
        "b1": (0.5 * b1.astype(np.float32)).reshape(1, -1).astype(bf),
        "w2t": np.ascontiguousarray((0.5 * W2.astype(np.float32)).T).astype(bf),
        "b2": (0.5 * b2.astype(np.float32)).reshape(1, -1).astype(bf),
        "wot": np.ascontiguousarray(
            np.pad((0.5 * Wo.astype(np.float32)).T, ((0, 0), (0, OP - O)))).astype(bf),
        "bo": np.pad(0.5 * bo.astype(np.float32), (0, OP - O)).reshape(1, -1).astype(bf),
    }
    in_maps = []
    for c in range(NCORES):
        xs = x[c * BS:(c + 1) * BS]                      # (BS, NIN, T)
        xs = np.ascontiguousarray(xs.transpose(1, 2, 0)).astype(bf)  # (NIN, T, BS)
        in_maps.append({**base, "x": xs})
    return in_maps


def kernel(x, W1, b1, W2, b2, Wo, bo, _trace=False, _trace_kwargs=None):
    x = np.asarray(x)
    T = x.shape[2]
    nc = _get_nc(T, BS)
    in_maps = _prep_in_maps(np.asarray(x, np.float32), np.asarray(W1),
                            np.asarray(b1), np.asarray(W2), np.asarray(b2),
                            np.asarray(Wo), np.asarray(bo))
    kw = {}
    if _trace:
        kw = {"trace": True, **(_trace_kwargs or {})}
    r = bass_utils.run_bass_kernel_spmd(nc, in_maps, core_ids=list(range(NCORES)), **kw)
    outs = []
    for c in range(NCORES):
        accv = r.results[c]["acc"]                       # (OP, BS) fp32
        outs.append(accv[:O, :].T / np.float32(T))
    out = np.concatenate(outs, axis=0).astype(np.float32)
    kernel._last_results = r
    return out
